# revision 100
# baseline (speedup 1.0000x reference)
"""Trainium2 Bass kernel for a pre-norm transformer block (nn_Block_38843684225792).

Full inputs -> full outputs. Sharding: data-parallel over batch, one batch
element per NeuronCore (8 cores). Inside each core the block is computed
channel-major (channels on SBUF partitions) so every matmul contracts over
the partition dim without extra transposes; x is transposed once on entry
and the result transposed back on exit via PE transposes.

Shapes (per core): x [1024, 768], heads=12, hd=64, mlp hidden=3072.
"""

import os
import sys

sys.path.insert(0, "/opt/trn_rl_repo")

import numpy as np

import concourse.bass as bass
import concourse.tile as tile
from concourse import bacc, mybir
from concourse.bass_utils import run_bass_kernel_spmd
from concourse.masks import make_identity

F32 = mybir.dt.float32
F32R = mybir.dt.float32r
BF16 = mybir.dt.bfloat16
AF = mybir.ActivationFunctionType

N_CORES = 8
S = 1024          # sequence length per core
C = 768           # model dim
H = 12            # heads
HD = 64           # head dim
HID = 3072        # mlp hidden
NCH = C // 128    # 6 channel chunks
NT = S // 128     # 8 token chunks
NFH = HID // 128  # 24 hidden chunks
EPS = 1e-5
ATT_SCALE = HD ** -0.5  # 0.125

_cached = {}



def _ln(nc, ps, work, src_tiles, dst, g_ap, b_ap, ones_col, ones_bf, eps_ap,
        tagp="", mid_hook=None):
    """Channel-major LayerNorm, both 512-token halves.

    Sums run per half (optionally with mid_hook() emitted between them);
    the serial stats chains of the two halves are interleaved so Ln/Exp
    table loads happen once and chain latencies overlap.
    """
    sums = []
    for n in (0, 1):
        sl = slice(512 * n, 512 * (n + 1))
        # single [1,1024] sums tile on the (otherwise idle) "po" psum tag:
        # sumx in cols 0:512, sumx2 in cols 512:1024.
        psum2 = ps.tile([1, S], F32, tag="po", bufs=1, name=f"ln_sums{tagp}{n}")
        for c in range(NCH):
            x2 = work.tile([128, 512], BF16, tag="x2", bufs=2,
                           name=f"ln_x2_{tagp}{n}{c}")
            nc.vector.tensor_mul(out=x2, in0=src_tiles[c].bitcast(F32)[:, sl],
                                 in1=src_tiles[c].bitcast(F32)[:, sl])
            nc.tensor.matmul(psum2[:, 0:512], ones_col, src_tiles[c][:, sl],
                             start=(c == 0), stop=(c == NCH - 1))
            nc.tensor.matmul(psum2[:, 512:1024], ones_bf, x2,
                             start=(c == 0), stop=(c == NCH - 1))
        mu = work.tile([1, 512], F32, tag="stats", bufs=4, name=f"ln_mu{tagp}{n}")
        nc.scalar.mul(out=mu, in_=psum2[:, 0:512], mul=1.0 / C)
        ex2 = work.tile([1, 512], F32, tag="stats", bufs=4,
                        name=f"ln_ex2{tagp}{n}")
        nc.scalar.mul(out=ex2, in_=psum2[:, 512:1024], mul=1.0 / C)
        sums.append((mu, ex2))
        if n == 0 and mid_hook is not None:
            mid_hook()
    # combined [1,1024] var/rstd rows (half n in cols 512n:512n+512) so Ln
    # and Exp are single instructions -> single table load each. They live
    # in the "r_raw" tag buffers, which are only used mid-attention.
    var01 = work.tile([1, S], F32, tag="r_raw", bufs=2, name=f"ln_var{tagp}")
    for n in (0, 1):
        mu, ex2 = sums[n]
        v = var01[:, 512 * n:512 * (n + 1)]
        nc.vector.tensor_mul(out=v, in0=mu, in1=mu)
        nc.vector.tensor_sub(out=v, in0=ex2, in1=v)
    nc.scalar.activation(out=var01, in_=var01, func=AF.Ln, bias=eps_ap,
                         scale=1.0)
    rstd01 = work.tile([1, S], F32, tag="r_raw", bufs=2, name=f"ln_rstd{tagp}")
    nc.scalar.activation(out=rstd01, in_=var01, func=AF.Exp, bias=0.0,
                         scale=-0.5)
    rstd = [rstd01[:, 0:512], rstd01[:, 512:1024]]
    muR01 = work.tile([1, S], F32, tag="muR", bufs=1, name=f"ln_muR{tagp}")
    for n in (0, 1):
        sl = slice(512 * n, 512 * (n + 1))
        mu, _ = sums[n]
        muR = muR01[:, 512 * n:512 * (n + 1)]
        nc.vector.tensor_mul(out=muR, in0=mu, in1=rstd[n])
        # broadcasts land in the halves of the attention-idle "pr" tile;
        # half 1's WAR on half 0's applies just serializes the halves.
        pr2 = work.tile([128, S], F32, tag="pr", bufs=1,
                        name=f"ln_pr{tagp}{n}")
        b_rstd = pr2[:, 0:512]
        nc.gpsimd.partition_broadcast(b_rstd, rstd[n])
        b_muR = pr2[:, 512:1024]
        nc.gpsimd.partition_broadcast(b_muR, muR)
        for c in range(NCH):
            t = work.tile([128, 512], F32, tag="tmp", bufs=2,
                          name=f"ln_t_{tagp}{n}{c}")
            nc.vector.tensor_mul(out=t, in0=src_tiles[c].bitcast(F32)[:, sl],
                                 in1=b_rstd)
            nc.vector.tensor_sub(out=t, in0=t, in1=b_muR)
            nc.scalar.activation(out=dst[c][:, sl], in_=t, func=AF.Identity,
                                 bias=b_ap[:, c:c + 1], scale=g_ap[:, c:c + 1])


def build():
    nc = bacc.Bacc(None, target_bir_lowering=False, debug=False)
    # larger SWDGE descriptor ring: fc2 half-tiles generate 1536 descriptors
    nc.dynamic_dma_scratch_size = 65536
    x_d = nc.declare_dram_parameter("x", [S, C], F32, isOutput=False)
    ln1_g_d = nc.declare_dram_parameter("ln1_g", [C], F32, isOutput=False)
    ln1_b_d = nc.declare_dram_parameter("ln1_b", [C], F32, isOutput=False)
    w_qkv_d = nc.declare_dram_parameter("w_qkv", [C, 3 * C], F32, isOutput=False)
    w_proj_d = nc.declare_dram_parameter("w_proj", [C, C], F32, isOutput=False)
    b_proj_d = nc.declare_dram_parameter("b_proj", [C], F32, isOutput=False)
    ln2_g_d = nc.declare_dram_parameter("ln2_g", [C], F32, isOutput=False)
    ln2_b_d = nc.declare_dram_parameter("ln2_b", [C], F32, isOutput=False)
    w_fc1_d = nc.declare_dram_parameter("w_fc1", [C, HID], F32, isOutput=False)
    b_fc1_d = nc.declare_dram_parameter("b_fc1", [HID], F32, isOutput=False)
    w_fc2_d = nc.declare_dram_parameter("w_fc2", [HID, C], F32, isOutput=False)
    b_fc2_d = nc.declare_dram_parameter("b_fc2", [C], F32, isOutput=False)
    out_d = nc.declare_dram_parameter("out", [S, C], F32, isOutput=True)

    from contextlib import ExitStack
    with tile.TileContext(nc) as tc, ExitStack() as ctx:
        consts = ctx.enter_context(tc.tile_pool(name="consts", bufs=1))
        arena = ctx.enter_context(tc.tile_pool(name="arena", bufs=1))
        work = ctx.enter_context(tc.tile_pool(name="work", bufs=1))
        ps = ctx.enter_context(tc.tile_pool(name="ps", bufs=2, space="PSUM"))

        # ---------------- constants ----------------
        ident = consts.tile([128, 128], F32, name="ident")
        make_identity(nc, ident)
        ident_r = consts.tile([128, 128], F32R, name="ident_r")
        nc.vector.tensor_copy(out=ident_r, in_=ident)
        ones_f32 = consts.tile([128, 1], F32, name="ones_f32")
        nc.vector.memset(ones_f32, 1.0)
        ones_col = consts.tile([128, 1], F32R, name="ones_col")
        nc.vector.tensor_copy(out=ones_col, in_=ones_f32)
        ones_bf = consts.tile([128, 1], BF16, name="ones_bf")
        nc.vector.tensor_copy(out=ones_bf, in_=ones_f32)
        eps_ap = consts.tile([1, 1], F32, name="eps_ap")
        nc.vector.memset(eps_ap, EPS)



        def load_chanvec(dram_t, name, width):
            # contiguous load + PE transpose: a direct "(o p) -> p o" DMA
            # would have 4-byte elements and hog a DGE queue for ~10us.
            stage = work.tile([width, 128], F32, tag="cvst", bufs=2,
                              name=f"{name}_st")
            nc.scalar.dma_start(out=stage,
                                in_=dram_t.ap().rearrange("(o p) -> o p", p=128))
            pst = ps.tile([128, width], F32, tag="ps3", bufs=3,
                          name=f"{name}_ps")
            nc.tensor.transpose(pst, stage, ident[0:width, 0:width])
            t = consts.tile([128, width], F32, name=name)
            nc.vector.tensor_copy(out=t, in_=pst)
            return t

        # ---------------- stage A + LN1, interleaved per token half ------
        # Token half n needs only transposes of token chunks 4n..4n+3, so the
        # second batch of transposes runs under half 0's serial stats chain.
        xT = [arena.tile([128, S], F32R, tag=f"xT{c}", name=f"xT{c}")
              for c in range(NCH)]
        # hT in bf16: all downstream matmuls (v, qk, fc1) run bf16 x bf16.
        hT = [arena.tile([128, S], BF16, tag=f"hT{c}", name=f"hT{c}")
              for c in range(NCH)]

        def transpose_in(a):
            x_sb = work.tile([128, C], F32, tag="x_sb", bufs=4, name=f"x_sb{a}")
            # chunk 0's first transpose only needs the first 128 columns
            cols = ((0, 128), (128, 384), (384, 768)) if a == 0 else \
                ((0, 384), (384, 768))
            # second batch all on sync: the scalar queue then only carries
            # chanvec loads + the alternating copies
            eng = nc.sync if (a % 2 == 0 or a >= 4) else nc.scalar
            for lo, hi in cols:
                eng.dma_start(out=x_sb[:, lo:hi],
                              in_=x_d.ap()[128 * a:128 * (a + 1), lo:hi])
            for c in range(NCH):
                pst = ps.tile([128, 128], F32, tag="ps3", bufs=3,
                              name=f"ptx{a}_{c}")
                nc.tensor.transpose(pst, x_sb[:, 128 * c:128 * (c + 1)], ident)
                dst = xT[c][:, 128 * a:128 * (a + 1)]
                # alternate copy engines: DVE is the early-phase bottleneck
                if c % 2 == 0:
                    nc.vector.tensor_copy(out=dst, in_=pst)
                else:
                    nc.scalar.activation(out=dst, in_=pst, func=AF.Identity,
                                         bias=0.0, scale=1.0)

        # generic per-half LN machinery (used for LN1 and LN2): stats into
        # a [1,1024] psum row (sumx | sumx2), then a chain spread across
        # engines -- DVE only reads psum and runs the reciprocal; var/muR
        # run on gpsimd so the DVE queue isn't head-of-line blocked while
        # Sqrt round-trips through the scalar engine.
        def ln_stats(n, src_tiles, tagp):
            sl = slice(512 * n, 512 * (n + 1))
            psum2 = ps.tile([1, S], F32, tag="po", bufs=1,
                            name=f"lnsum{tagp}{n}")
            for c in range(NCH):
                x2 = work.tile([128, 512], BF16, tag="x2", bufs=2,
                               name=f"lnx2{tagp}{n}{c}")
                nc.vector.tensor_mul(out=x2,
                                     in0=src_tiles[c].bitcast(F32)[:, sl],
                                     in1=src_tiles[c].bitcast(F32)[:, sl])
                nc.tensor.matmul(psum2[:, 0:512], ones_col,
                                 src_tiles[c][:, sl],
                                 start=(c == 0), stop=(c == NCH - 1))
                nc.tensor.matmul(psum2[:, 512:1024], ones_bf, x2,
                                 start=(c == 0), stop=(c == NCH - 1))
            return psum2

        def ln_chain(n, psum2, src_tiles, dst, g_ap, b_ap, tagp):
            sl = slice(512 * n, 512 * (n + 1))
            mu = work.tile([1, 512], F32, tag="stats", bufs=4,
                           name=f"lnmu{tagp}{n}")
            nc.vector.tensor_scalar_mul(out=mu, in0=psum2[:, 0:512],
                                        scalar1=1.0 / C)
            ex2 = work.tile([1, 512], F32, tag="stats", bufs=4,
                            name=f"lnex2{tagp}{n}")
            nc.vector.tensor_scalar_mul(out=ex2, in0=psum2[:, 512:1024],
                                        scalar1=1.0 / C)
            # var/muR on DVE: gpsimd tensor ops would force ~7us Q7 library
            # swaps between the broadcast and tensor payloads.
            var = work.tile([1, 512], F32, tag="r_raw", bufs=2,
                            name=f"lnvar{tagp}{n}")
            nc.vector.tensor_mul(out=var, in0=mu, in1=mu)
            nc.vector.tensor_sub(out=var, in0=ex2, in1=var)
            rstd = work.tile([1, 512], F32, tag="r_raw", bufs=2,
                             name=f"lnrstd{tagp}{n}")
            nc.scalar.activation(out=rstd, in_=var, func=AF.Sqrt,
                                 bias=eps_ap, scale=1.0)
            nc.vector.reciprocal_approx_fast(out=rstd, in_=rstd)
            muR = work.tile([1, 512], F32, tag="stats", bufs=4,
                            name=f"lnmuR{tagp}{n}")
            nc.vector.tensor_mul(out=muR, in0=mu, in1=rstd)
            pr2 = work.tile([128, S], F32, tag="pr", bufs=1,
                            name=f"lnpr{tagp}{n}")
            b_rstd = pr2[:, 0:512]
            nc.gpsimd.partition_broadcast(b_rstd, rstd)
            b_muR = pr2[:, 512:1024]
            nc.gpsimd.partition_broadcast(b_muR, muR)
            for c in range(NCH):
                t = work.tile([128, 512], F32, tag="tmp", bufs=2,
                              name=f"lnt{tagp}{n}{c}")
                nc.vector.tensor_mul(out=t,
                                     in0=src_tiles[c].bitcast(F32)[:, sl],
                                     in1=b_rstd)
                nc.vector.tensor_sub(out=t, in0=t, in1=b_muR)
                nc.scalar.activation(out=dst[c][:, sl], in_=t,
                                     func=AF.Identity,
                                     bias=b_ap[:, c:c + 1],
                                     scale=g_ap[:, c:c + 1])

        for a in range(4):
            transpose_in(a)
        g1 = load_chanvec(ln1_g_d, "g1", NCH)
        b1 = load_chanvec(ln1_b_d, "b1", NCH)
        g2 = load_chanvec(ln2_g_d, "g2", NCH)
        b2 = load_chanvec(ln2_b_d, "b2", NCH)
        bp = load_chanvec(b_proj_d, "bp", NCH)
        bf1 = load_chanvec(b_fc1_d, "bf1", NFH)
        bf2 = load_chanvec(b_fc2_d, "bf2", NCH)

        # pool open order is LIFO-constrained: wqk and wpp (kernel-lifetime)
        # open before the short-lived wvp; wmlp opens after wvp closes and
        # reuses its space.
        wqkp = ctx.enter_context(tc.tile_pool(name="wqk", bufs=1))
        wpp = ctx.enter_context(tc.tile_pool(name="wpp", bufs=1))

        # v weights early on the gpsimd SWDGE queue (casting DMA f32->bf16),
        # before LN1's broadcasts so those aren't queued behind the descgen.
        v_aug = arena.tile([128, NT, H, 66], BF16, tag="v_aug", name="v_aug")
        nc.vector.memset(v_aug[:, :, :, 64:65], 1.0)
        wvp_cm = tc.tile_pool(name="wvp", bufs=1)
        wvp = wvp_cm.__enter__()
        wv_t = []
        for n in range(2):
            w = wvp.tile([128, NCH, 384], BF16, tag=f"wv{n}", bufs=1,
                         name=f"wv{n}")
            nc.gpsimd.dma_start(
                out=w,
                in_=w_qkv_d.ap()[:, 1536 + 384 * n:1536 + 384 * (n + 1)]
                .rearrange("(ko ki) m -> ki ko m", ki=128))
            wv_t.append(w)

        # ---- LN1 + v, token-half pipelined:
        # tr-h0 | stats-h0 | [chain-h0 off-PE] tr-h1 | stats-h1 | v-h0 |
        # [chain-h1 off-PE] v-h1 | qk/wp loads | qkT(0) | attention
        def v_half(th):
            for n in range(2):  # halves of v channels (heads 6n..6n+5)
                for mt in range(4 * th, 4 * th + 4):
                    pv = ps.tile([128, 384], F32, tag="ps3", bufs=3,
                                 name=f"pv{n}_{mt}")
                    for ko in range(NCH):
                        nc.tensor.matmul(
                            pv, hT[ko][:, 128 * mt:128 * (mt + 1)],
                            wv_t[n][:, ko, :],
                            start=(ko == 0), stop=(ko == NCH - 1))
                    pv3 = pv.rearrange("p (j d) -> p j d", d=HD)
                    dst = v_aug[:, mt, 6 * n:6 * n + 6, 0:64]
                    if mt % 2 == 0:
                        nc.vector.tensor_copy(out=dst, in_=pv3)
                    else:
                        nc.scalar.activation(out=dst, in_=pv3,
                                             func=AF.Identity, bias=0.0,
                                             scale=1.0)

        psum_l1h0 = ln_stats(0, xT, "l1")
        for a in range(4, 8):
            transpose_in(a)
        ln_chain(0, psum_l1h0, xT, hT, g1, b1, "l1")
        psum_l1h1 = ln_stats(1, xT, "l1")
        v_half(0)
        ln_chain(1, psum_l1h1, xT, hT, g1, b1, "l1")

        # all q/k weights as bf16 casting DMAs (4 descgens: 3 pairs each),
        # queued after LN1's chain ops
        qk_big = {}
        for which, base in (("k", 768), ("q", 0)):
            for h in range(2):
                w = wqkp.tile([128, NCH, 384], BF16, tag=f"w{which}{h}",
                              bufs=1, name=f"w{which}{h}")
                nc.gpsimd.dma_start(
                    out=w,
                    in_=w_qkv_d.ap()[:, base + 384 * h:base + 384 * (h + 1)]
                    .rearrange("(ko ki) m -> ki ko m", ki=128))
                qk_big[(which, h)] = w
        wqk_all = []
        for p in range(6):
            sl_m = slice(128 * (p % 3), 128 * (p % 3) + 128)
            wqk_all.append((qk_big[("k", p // 3)][:, :, sl_m],
                            qk_big[("q", p // 3)][:, :, sl_m]))

        # proj weights, bf16, all six resident (no ring hazards across the
        # two proj half-passes); descgen queued behind the qk loads, long
        # before the attention-era broadcasts need the gpsimd queue.
        wp_tiles = []
        for mc in range(NCH):
            wp = wpp.tile([128, NCH, 128], BF16, tag=f"wp{mc}", bufs=1,
                          name=f"wp{mc}")
            nc.gpsimd.dma_start(
                out=wp,
                in_=w_proj_d.ap()[:, 128 * mc:128 * (mc + 1)]
                .rearrange("(ko ki) m -> ki ko m", ki=128))
            wp_tiles.append(wp)

        v_half(1)
        wvp_cm.__exit__(None, None, None)
        wmlp = ctx.enter_context(tc.tile_pool(name="wmlp", bufs=1))
        # first fc1 weight tiles allocated NOW: the wmlp pool guard snapshots
        # engine counters at allocation, so allocating early releases the
        # freed-wvp space guard early and the transfers land pre-attention.
        # mlp weight loaders: gpsimd casting DMAs (f32 DRAM -> bf16 SBUF).
        # Fresh wpp tags, so early emission is safe; the gpsimd queue orders
        # them after the attention-era broadcasts.
        def load_w1(j):
            # one tile covers fc1 output chunks 2j and 2j+1; lives in wpp
            # (fresh addresses, no freed-space guard on the early loads)
            w1 = wpp.tile([128, NCH, 256], BF16, tag="w1", bufs=3,
                          name=f"w1_{j}")
            nc.gpsimd.dma_start(
                out=w1,
                in_=w_fc1_d.ap()[:, 256 * j:256 * (j + 1)]
                .rearrange("(ko ki) m -> ki ko m", ki=128))
            return w1

        # fc2 weight halves are streamed twice (once per token-half pass);
        # needs the bigger SWDGE ring (dynamic_dma_scratch_size).
        def load_w2h(mc, half, n):
            t = wmlp.tile([128, NFH // 2, 128], BF16, tag="w2", bufs=3,
                          name=f"w2_{n}_{mc}_{half}")
            nc.gpsimd.dma_start(
                out=t,
                in_=w_fc2_d.ap()[:, 128 * mc:128 * (mc + 1)]
                .rearrange("(f ki) m -> ki f m", ki=128)
                [:, 12 * half:12 * (half + 1), :])
            return t

        w1_early = [load_w1(0), load_w1(1), load_w1(2)]

        # ---------------- stages C1+D: qk^T and attention per head pair --
        # attnT in bf16 (proj runs bf16 x bf16)
        attnT = [arena.tile([128, S], BF16, tag=f"attnT{c}", name=f"attnT{c}")
                 for c in range(NCH)]
        if True:
            def emit_qkT(p, wk, wq):
                # k first: its cast overlaps the q matmuls. q's cast is split
                # per half so the next pair's score matmuls start sooner.
                pqk_k = ps.tile([128, S], F32, tag="ps3", bufs=3,
                                name=f"pqkk{p}")
                for n in range(2):
                    sl = slice(512 * n, 512 * (n + 1))
                    for ko in range(NCH):
                        nc.tensor.matmul(pqk_k[:, sl], wk[:, ko, :],
                                         hT[ko][:, sl],
                                         start=(ko == 0), stop=(ko == NCH - 1))
                kT = arena.tile([128, S], BF16, tag="kT", bufs=2, name=f"kT{p}")
                # kT cast on scalar: runs in parallel with the q casts on
                # DVE, so the next pair's score matmuls start sooner
                nc.scalar.activation(out=kT, in_=pqk_k, func=AF.Identity,
                                     bias=0.0, scale=1.0)
                pqk_q = ps.tile([128, S], F32, tag="ps3", bufs=3,
                                name=f"pqkq{p}")
                qT = arena.tile([128, S], BF16, tag="qT", bufs=2, name=f"qT{p}")
                for n in range(2):
                    sl = slice(512 * n, 512 * (n + 1))
                    for ko in range(NCH):
                        nc.tensor.matmul(pqk_q[:, sl], wq[:, ko, :],
                                         hT[ko][:, sl],
                                         start=(ko == 0), stop=(ko == NCH - 1))
                    nc.vector.tensor_copy(out=qT[:, sl], in_=pqk_q[:, sl])
                return qT, kT

            def emit_S(hh, kc, qTp, kTp):
                base = 64 * (hh % 2)
                kcs = slice(128 * kc, 128 * (kc + 1))
                pS = ps.tile([128, S], F32, tag="ps3", bufs=3,
                             name=f"pS{hh}_{kc}")
                for n in range(2):
                    sl = slice(512 * n, 512 * (n + 1))
                    nc.tensor.matmul(pS[:, sl], kTp[base:base + 64, kcs],
                                     qTp[base:base + 64, sl])
                expS = work.tile([128, S], BF16, tag="expS", bufs=3,
                                 name=f"expS{hh}_{kc}")
                nc.scalar.activation(out=expS, in_=pS, func=AF.Exp,
                                     bias=0.0, scale=ATT_SCALE)
                return expS

            def normalize(hh, po, fast_den=False):
                # normalize rows by the ones-column row sums (all off-PE,
                # except fast_den which spends one PE matmul to move the
                # denominator row to partition 0 without a DMA round-trip)
                o_sb = work.tile([65, S], F32R, tag="o_sb", bufs=2,
                                 name=f"o_sb{hh}")
                nc.vector.tensor_copy(out=o_sb, in_=po[0:65, :])
                r_rec = work.tile([1, S], F32, tag="r_raw", bufs=2,
                                  name=f"r_rec{hh}")
                r_raw = work.tile([1, S], F32, tag="r_raw", bufs=2,
                                  name=f"r_raw{hh}")
                nc.sync.dma_start(out=r_raw,
                                  in_=o_sb[64:65, :].bitcast(F32))
                nc.vector.reciprocal_approx_fast(out=r_rec, in_=r_raw)
                pr_sb = work.tile([128, S], F32, tag="pr", bufs=1,
                                  name=f"pr{hh}")
                nc.gpsimd.partition_broadcast(pr_sb, r_rec)
                c2 = hh // 2
                if hh % 2 == 0:
                    nc.vector.tensor_mul(out=attnT[c2][0:64, :],
                                         in0=o_sb[0:64, :].bitcast(F32),
                                         in1=pr_sb[0:64, :])
                else:
                    o2 = work.tile([64, S], BF16, tag="o_sb2", bufs=2,
                                   name=f"o2_{hh}")
                    nc.vector.tensor_mul(out=o2,
                                         in0=o_sb[0:64, :].bitcast(F32),
                                         in1=pr_sb[0:64, :])
                    nc.sync.dma_start(out=attnT[c2][64:128, :], in_=o2)

            # software-pipelined pair loop, two steps deep: the S matmuls of
            # steps i+1 and i+2 run while step i+1's exp is on the scalar
            # engine, so the PE never parks behind exp latency.
            qkT_next = emit_qkT(0, *wqk_all[0])
            for p in range(6):  # head pair (2p, 2p+1)
                qTp, kTp = qkT_next
                first, second = (2 * p, 2 * p + 1) if p < 5 else \
                    (2 * p + 1, 2 * p)
                steps = [(first, kc) for kc in range(NT)] + \
                        [(second, kc) for kc in range(NT)]
                exps = {0: emit_S(*steps[0], qTp, kTp),
                        1: emit_S(*steps[1], qTp, kTp)}
                po = {first: ps.tile([128, S], F32, tag="po", bufs=1,
                                     name=f"po{first}")}
                for i, (hh, kc) in enumerate(steps):
                    if i + 2 < len(steps):
                        exps[i + 2] = emit_S(*steps[i + 2], qTp, kTp)
                    elif i == len(steps) - 2 and p < 5:
                        # pair p+1's qk^T fills the last two exps' latency
                        qkT_next = emit_qkT(p + 1, *wqk_all[p + 1])
                    if hh == second and kc == 0:
                        po[second] = ps.tile([128, S], F32, tag="po", bufs=1,
                                             name=f"po{second}")
                    for n in range(2):
                        sl = slice(512 * n, 512 * (n + 1))
                        nc.tensor.matmul(
                            po[hh][0:65, sl], v_aug[:, kc, hh, 0:65],
                            exps[i][:, sl],
                            start=(kc == 0), stop=(kc == NT - 1))
                    if hh == first and kc == NT - 1:
                        normalize(first, po[first])
                normalize(second, po[second], fast_den=(p == 5))

        # ---------------- stages E+F: proj + residual + LN2, per half ----
        # Half 0's LN2 stats accumulate as proj chunks complete, so its
        # rstd chain and applies hide under proj half 1 and fc1 can start
        # almost immediately after the proj matmuls end.
        out1T = [arena.tile([128, S], F32R, tag=f"out1T{mc}",
                            name=f"out1T{mc}") for mc in range(NCH)]
        h2T = [arena.tile([128, S], BF16, tag=f"hT{c}", name=f"h2T{c}")
               for c in range(NCH)]

        psum_l2 = {}
        for n in range(2):
            sl = slice(512 * n, 512 * (n + 1))
            psum2 = ps.tile([1, S], F32, tag="po", bufs=1, name=f"l2sums{n}")
            for mc in range(NCH):
                wp = wp_tiles[mc]
                py = ps.tile([128, 512], F32, tag="ps3", bufs=3,
                             name=f"py{n}_{mc}")
                for ko in range(NCH):
                    nc.tensor.matmul(py, wp[:, ko, :], attnT[ko][:, sl],
                                     start=(ko == 0), stop=False)
                # residual folded into psum via identity matmul: the psum
                # then drains straight through the scalar engine (no DVE
                # add on the py ring's critical path)
                nc.tensor.matmul(py, ident_r, xT[mc][:, sl],
                                 start=False, stop=True)
                nc.scalar.activation(out=out1T[mc][:, sl], in_=py,
                                     func=AF.Identity,
                                     bias=bp[:, mc:mc + 1], scale=1.0)
                x2 = work.tile([128, 512], BF16, tag="x2", bufs=2,
                               name=f"l2x2_{n}{mc}")
                nc.vector.tensor_mul(out=x2,
                                     in0=out1T[mc].bitcast(F32)[:, sl],
                                     in1=out1T[mc].bitcast(F32)[:, sl])
                nc.tensor.matmul(psum2[:, 0:512], ones_col,
                                 out1T[mc][:, sl],
                                 start=(mc == 0), stop=(mc == NCH - 1))
                nc.tensor.matmul(psum2[:, 512:1024], ones_bf, x2,
                                 start=(mc == 0), stop=(mc == NCH - 1))
            psum_l2[n] = psum2
            # chain emitted immediately: its Sqrt lands at the scalar
            # queue's head (before half 1's out1T identities), and with the
            # residual folded into psum there is no DVE drain to block.
            ln_chain(n, psum2, out1T, h2T, g2, b2, "l2")

        # ---------------- stages G+H: MLP (bf16) + stores per half -------
        if True:
            w1_tiles = w1_early
            # fc1 + gelu: a1 tile j ([128, 2048] bf16) holds hidden chunks
            # (2j cols 0:1024, 2j+1 cols 1024:2048).
            a1 = []
            for j in range(12):
                tag = f"xT{j}" if j < 6 else f"attnT{j - 6}"
                a1.append(arena.tile([128, 2 * S], BF16, tag=tag,
                                     name=f"a1_{j}"))

            def fc1_half(mc, n):
                w1 = w1_tiles[mc // 2]
                ms = slice(128 * (mc % 2), 128 * (mc % 2) + 128)
                sl = slice(512 * n, 512 * (n + 1))
                pg = ps.tile([128, 512], F32, tag="ps3", bufs=3,
                             name=f"pg{mc}_{n}")
                for ko in range(NCH):
                    nc.tensor.matmul(pg, w1[:, ko, ms], h2T[ko][:, sl],
                                     start=(ko == 0), stop=(ko == NCH - 1))
                dst = a1[mc // 2][:, S * (mc % 2) + 512 * n:
                                  S * (mc % 2) + 512 * (n + 1)]
                nc.scalar.activation(out=dst, in_=pg, func=AF.Gelu,
                                     bias=bf1[:, mc:mc + 1], scale=1.0)

            # the first six chunks run half 0 before half 1, giving the
            # PE work while half 1's LN2 chain + applies finish.
            for mc in range(6):
                fc1_half(mc, 0)
            for mc in range(NFH):
                if mc % 2 == 0 and mc // 2 + 3 < NFH // 2:
                    w1_tiles.append(load_w1(mc // 2 + 3))
                if mc < 6:
                    fc1_half(mc, 1)
                else:
                    fc1_half(mc, 0)
                    fc1_half(mc, 1)

            # fc2 + bias + residual per token half; after half 0 the output
            # transposes/stores for token chunks 0..3 run while half 1's
            # fc2 matmuls proceed.
            def transpose_out(a):
                o_out = work.tile([128, C], F32, tag="x_sb", bufs=4,
                                  name=f"o_out{a}")
                for mc in range(NCH):
                    pst = ps.tile([128, 128], F32, tag="ps3", bufs=3,
                                  name=f"pto{a}_{mc}")
                    nc.tensor.transpose(
                        pst, out1T[mc].bitcast(F32)[:, 128 * a:128 * (a + 1)],
                        ident)
                    dst = o_out[:, 128 * mc:128 * (mc + 1)]
                    if mc % 2 == 0:
                        nc.vector.tensor_copy(out=dst, in_=pst)
                    else:
                        nc.scalar.activation(out=dst, in_=pst,
                                             func=AF.Identity, bias=0.0,
                                             scale=1.0)
                nc.sync.dma_start(out=out_d.ap()[128 * a:128 * (a + 1), :],
                                  in_=o_out)

            w2_tiles = {}
            for n in range(2):
                for pre in (0, 1):
                    w2_tiles[(pre, 0)] = load_w2h(pre, 0, n)
                    w2_tiles[(pre, 1)] = load_w2h(pre, 1, n)
                sl = slice(512 * n, 512 * (n + 1))
                for mc in range(NCH):
                    if mc + 2 < NCH:
                        w2_tiles[(mc + 2, 0)] = load_w2h(mc + 2, 0, n)
                        w2_tiles[(mc + 2, 1)] = load_w2h(mc + 2, 1, n)
                    py2 = ps.tile([128, 512], F32, tag="ps3", bufs=3,
                                  name=f"py2_{n}_{mc}")
                    for f in range(NFH):
                        wt = w2_tiles[(mc, f // 12)]
                        rhs = a1[f // 2][:, S * (f % 2) + 512 * n:
                                         S * (f % 2) + 512 * (n + 1)]
                        nc.tensor.matmul(py2, wt[:, f % 12, :], rhs,
                                         start=(f == 0), stop=(f == NFH - 1))
                    t = work.tile([128, 512], F32, tag="tmp", bufs=2,
                                  name=f"ht{n}{mc}")
                    nc.scalar.activation(out=t, in_=py2, func=AF.Identity,
                                         bias=bf2[:, mc:mc + 1], scale=1.0)
                    nc.vector.tensor_add(out=out1T[mc][:, sl],
                                         in0=out1T[mc].bitcast(F32)[:, sl],
                                         in1=t)
                # ---------------- stage I: transpose back + store --------
                for a in range(4 * n, 4 * n + 4):
                    transpose_out(a)

    nc.compile()
    return nc


def _get_nc():
    if "nc" not in _cached:
        _cached["nc"] = build()
    return _cached["nc"]


def kernel(**inputs):
    nc = _get_nc()
    x = np.ascontiguousarray(np.asarray(inputs["x"], dtype=np.float32))
    weights = {
        k: np.ascontiguousarray(np.asarray(inputs[k], dtype=np.float32))
        for k in ("ln1_g", "ln1_b", "w_qkv", "w_proj", "b_proj",
                  "ln2_g", "ln2_b", "w_fc1", "b_fc1", "w_fc2", "b_fc2")
    }
    in_maps = [{"x": x[i], **weights} for i in range(N_CORES)]
    trace = bool(int(os.environ.get("BASS_KERNEL_TRACE", "0")))
    res = run_bass_kernel_spmd(nc, in_maps, list(range(N_CORES)), trace=trace)
    _cached["last_exec_time_ns"] = res.exec_time_ns
    out = np.stack([res.results[i]["out"] for i in range(N_CORES)], axis=0)
    return out.astype(np.float32)

